# revision 14
# baseline (speedup 1.0000x reference)
"""Trainium2 Bass kernel for nn_DynamicConv (per-pixel dynamic 5x5 conv, 8 heads).

Reference computation (per batch image b):
    f[i, j, :]  = sum_c x[b, c, i, j] * filt_w[c, :]          # (56,56,200)
    out[c, i, j] = sum_{kh,kw} xpad[c, i+kh, j+kw] * f[i, j, kh, kw, c//24]

Sharding: data-parallel over batch, but each core takes 2 images x one
28-column half of the width so that 112 of 128 SBUF partitions carry
(row, image) pairs: partition q = 2*row + img.  Compute-engine APs must
start at partition 0 (quad-aligned), so the five kh row shifts are
materialized as five separately-laid-out DRAM loads x_d0..x_d4
(x_dk[q, c, jp] = xpad[img, c, i+k, jp]); the kw shifts are free-dim
offsets.  The dynamic conv is then 25 broadcast-multiply + accumulate
passes on the vector engine, with the per-head filter value broadcast
across the 24 channels of its head via a step-0 AP.

Filter generation runs on the PE: per output column j, a (96ch x 112px)
slice of a channel-major copy of x is the stationary operand against
filt_w (96 x 200), accumulating the two 96-channel chunks in PSUM.
"""

import numpy as np

import concourse.bass as bass
import concourse.bacc as bacc
import concourse.mybir as mybir
import concourse.tile as tile
from concourse.bass_utils import run_bass_kernel_spmd

B, C, H, W = 8, 192, 56, 56
K, HEADS = 5, 8
CG = C // HEADS            # 24 channels per head
FCOLS = K * K * HEADS      # 200 filter-gen outputs per pixel
WH = 28                    # columns per core (half width)
JP = WH + 4                # padded columns held in SBUF
P_O = 2 * H                # 112 partitions carrying (row, img) pairs
JCH = 7                    # filter-gen j-columns per chunk
N_CORES = 8

F32 = mybir.dt.float32


VERSION = 3

F32R = mybir.dt.float32r
HHEADS = HEADS // 2        # heads per channel-half
NCH = 6                    # PSUM chunks per half
CHF = 96 * WH // NCH       # 448 fp32 per chunk = 16 channels x 28 cols


def build_nc(version=None):
    if version is None:
        version = VERSION
    nc = bacc.Bacc(None)

    xd_in = [
        nc.dram_tensor(f"x_d{k}", [P_O, C, JP], F32, kind="ExternalInput")
        for k in range(K)
    ]
    xg_in = nc.dram_tensor("x_gen", [96, 2, WH, P_O], F32, kind="ExternalInput")
    fw_in = nc.dram_tensor("fw_pk", [96, 2, FCOLS], F32, kind="ExternalInput")
    id_in = nc.dram_tensor("ident", [P_O, P_O], F32R, kind="ExternalInput")
    out_d = nc.dram_tensor("out_sbl", [P_O, C, WH], F32, kind="ExternalOutput")

    with tile.TileContext(nc) as tc:
        with (
            tc.tile_pool(name="big", bufs=1) as big,
            tc.tile_pool(name="sh", bufs=2) as sh,
            tc.tile_pool(name="ps_f", bufs=2, space="PSUM") as ps_f,
            tc.tile_pool(name="ps_a", bufs=NCH, space="PSUM") as ps_a,
        ):
            xd = [
                big.tile([P_O, C, JP], F32, tag=f"xd{k}", name=f"xd{k}")
                for k in range(K)
            ]
            fw_sb = big.tile([96, 2, FCOLS], F32)
            f_sb = big.tile([P_O, WH, K * K, HEADS], F32)
            acc = big.tile([P_O, C, WH], F32)
            ident = big.tile([P_O, P_O], F32R)

            nc.sync.dma_start(ident[:], id_in[:])
            for k in range(K):
                nc.sync.dma_start(xd[k][:], xd_in[k][:])
            nc.sync.dma_start(fw_sb[:], fw_in[:])

            # ---- filter generation: f[q, j, kk, h] = sum_c x[c, q, j] * fw[c, kk*8+h]
            for jc in range(WH // JCH):
                xg = sh.tile([96, 2, JCH, P_O], F32, tag="xgprod")
                nc.sync.dma_start(xg[:], xg_in[:, :, jc * JCH : (jc + 1) * JCH, :])
                for jl in range(JCH):
                    j = jc * JCH + jl
                    fps = ps_f.tile([P_O, K * K, HEADS], F32, tag="fps")
                    for ck in range(2):
                        nc.tensor.matmul(
                            fps[:],
                            xg[:, ck, jl, :],      # (96 ch, 112 px) stationary
                            fw_sb[:, ck, :],       # (96 ch, 200)
                            start=(ck == 0),
                            stop=(ck == 1),
                        )
                    nc.scalar.copy(f_sb[:, j, :, :], fps[:])

            if version == 1:
                # ---- 25 broadcast-mult + accumulate passes, all on DVE
                first = True
                for kh in range(K):
                    for kw in range(K):
                        kl = kh * K + kw
                        xin = xd[kh][:, :, kw : kw + WH]
                        xin4 = xin.rearrange("p (h g) j -> p h g j", h=HEADS)
                        fbc = (
                            f_sb[:, :, kl, :]
                            .transpose([0, 2, 1])
                            .unsqueeze(2)
                            .broadcast_to([P_O, HEADS, CG, WH])
                        )
                        if first:
                            prod = acc
                        else:
                            prod = sh.tile([P_O, C, WH], F32, tag="xgprod")
                        p4 = prod[:].rearrange("p (h g) j -> p h g j", h=HEADS)
                        nc.vector.tensor_mul(p4, xin4, fbc)
                        if not first:
                            nc.vector.tensor_add(acc[:], acc[:], prod[:])
                        first = False
            else:
                # ---- v2/v3: DVE computes the 25 products per channel-half;
                # the PE sums them with float32r identity matmuls
                # accumulating in PSUM (exact fp32), ACT evacuates.
                # v3 additionally gives GPSIMD a column share of the products.
                JS = WH if version == 2 else 18
                for hh in range(2):
                    c0 = hh * 96
                    accps = [
                        ps_a.tile([P_O, CHF], F32, tag="accps", name=f"accps{hh}_{b}")
                        for b in range(NCH)
                    ]
                    for kh in range(K):
                        for kw in range(K):
                            kl = kh * K + kw
                            xin = xd[kh][:, c0 : c0 + 96, kw : kw + WH]
                            xin4 = xin.rearrange("p (h g) j -> p h g j", h=HHEADS)
                            fbc = (
                                f_sb[:, :, kl, hh * HHEADS : (hh + 1) * HHEADS]
                                .transpose([0, 2, 1])
                                .unsqueeze(2)
                                .broadcast_to([P_O, HHEADS, CG, WH])
                            )
                            prod = sh.tile([P_O, 96, WH], F32R, tag="xgprod")
                            p4 = prod[:].rearrange("p (h g) j -> p h g j", h=HHEADS)
                            if JS >= WH:
                                nc.vector.tensor_mul(p4, xin4, fbc)
                            else:
                                nc.vector.tensor_mul(
                                    p4[:, :, :, :JS],
                                    xin4[:, :, :, :JS],
                                    fbc[:, :, :, :JS],
                                )
                                nc.gpsimd.tensor_mul(
                                    p4[:, :, :, JS:],
                                    xin4[:, :, :, JS:],
                                    fbc[:, :, :, JS:],
                                )
                            pflat = prod[:].rearrange("p c j -> p (c j)")
                            for b in range(NCH):
                                nc.tensor.matmul(
                                    accps[b][:],
                                    ident[:],
                                    pflat[:, b * CHF : (b + 1) * CHF],
                                    start=(kl == 0),
                                    stop=(kl == K * K - 1),
                                )
                    for b in range(NCH):
                        nc.scalar.copy(
                            acc[:, hh * 96 + b * 16 : hh * 96 + (b + 1) * 16, :],
                            accps[b][:].rearrange("p (c j) -> p c j", j=WH),
                        )

            nc.sync.dma_start(out_d[:], acc[:])

    return nc


def shard_inputs(x, filt_w):
    """Split full inputs into the 8 per-core input maps."""
    x = np.ascontiguousarray(np.asarray(x, dtype=np.float32))
    fw = np.ascontiguousarray(np.asarray(filt_w, dtype=np.float32))
    fw_pk = np.ascontiguousarray(fw.reshape(2, 96, FCOLS).transpose(1, 0, 2))

    in_maps = []
    for core in range(N_CORES):
        pair, jh = divmod(core, 2)
        xs = x[2 * pair : 2 * pair + 2]           # (2, C, 56, 56)
        xpad = np.zeros((2, C, H + 4, JP), np.float32)
        lo = jh * WH - 2                           # global col of jp=0
        s_lo, s_hi = max(lo, 0), min(lo + JP, W)
        xpad[:, :, 2 : 2 + H, s_lo - lo : s_lo - lo + (s_hi - s_lo)] = xs[
            :, :, :, s_lo:s_hi
        ]
        m = {"fw_pk": fw_pk, "ident": np.eye(P_O, dtype=np.float32)}
        for k in range(K):
            # x_dk[2*i+img, c, jp] = xpad[img, c, i+k, jp]
            m[f"x_d{k}"] = np.ascontiguousarray(
                xpad[:, :, k : k + H, :].transpose(2, 0, 1, 3).reshape(P_O, C, JP)
            )
        # channel-major copy for filter-gen: x_gen[c96, ck, j, 2*i+img]
        xs_half = xs[:, :, :, jh * WH : (jh + 1) * WH]  # (2, C, 56, 28)
        xg = xs_half.transpose(1, 3, 2, 0).reshape(C, WH, P_O)
        m["x_gen"] = np.ascontiguousarray(
            xg.reshape(2, 96, WH, P_O).transpose(1, 0, 2, 3)
        )
        in_maps.append(m)
    return in_maps


def unshard_output(results):
    """Reassemble the 8 per-core outputs into the full (B, C, H, W) tensor."""
    out = np.empty((B, C, H, W), np.float32)
    for core in range(N_CORES):
        pair, jh = divmod(core, 2)
        arr = np.asarray(results[core]["out_sbl"]).reshape(H, 2, C, WH)
        # arr[i, img, c, j] = out[2*pair+img, c, i, jh*28+j]
        out[2 * pair : 2 * pair + 2, :, :, jh * WH : (jh + 1) * WH] = arr.transpose(
            1, 2, 0, 3
        )
    return out


_NC_CACHE = None


def _get_nc():
    global _NC_CACHE
    if _NC_CACHE is None:
        _NC_CACHE = build_nc()
        if not _NC_CACHE.is_finalized():
            _NC_CACHE.finalize()
    return _NC_CACHE


def run(inputs, trace=False, **kwargs):
    """Run on the 8 NeuronCores; returns BassKernelResults."""
    in_maps = shard_inputs(inputs["x"], inputs["filt_w"])
    nc = _get_nc()
    return run_bass_kernel_spmd(
        nc, in_maps, core_ids=list(range(N_CORES)), trace=trace, **kwargs
    )


def kernel(x, filt_w):
    res = run({"x": x, "filt_w": filt_w})
    return unshard_output(res.results)


# revision 18
# speedup vs baseline: 1.5901x; 1.5901x over previous
"""Trainium2 Bass kernel for nn_DynamicConv (per-pixel dynamic 5x5 conv, 8 heads).

Reference computation (per batch image b):
    f[i, j, :]  = sum_c x[b, c, i, j] * filt_w[c, :]          # (56,56,200)
    out[c, i, j] = sum_{kh,kw} xpad[c, i+kh, j+kw] * f[i, j, kh, kw, c//24]

Sharding: data-parallel over batch, but each core takes 2 images x one
28-column half of the width so that 112 of 128 SBUF partitions carry
(row, image) pairs: partition q = 2*row + img.  Compute-engine APs must
start at partition 0 (quad-aligned), so the five kh row shifts are
materialized as five separately-laid-out DRAM loads x_d0..x_d4
(x_dk[q, c, jp] = xpad[img, c, i+k, jp]); the kw shifts are free-dim
offsets.  The dynamic conv is then 25 broadcast-multiply + accumulate
passes on the vector engine, with the per-head filter value broadcast
across the 24 channels of its head via a step-0 AP.

Filter generation runs on the PE: per output column j, a (96ch x 112px)
slice of a channel-major copy of x is the stationary operand against
filt_w (96 x 200), accumulating the two 96-channel chunks in PSUM.
"""

import numpy as np

import concourse.bass as bass
import concourse.bacc as bacc
import concourse.mybir as mybir
import concourse.tile as tile
from concourse.bass_utils import run_bass_kernel_spmd

B, C, H, W = 8, 192, 56, 56
K, HEADS = 5, 8
CG = C // HEADS            # 24 channels per head
FCOLS = K * K * HEADS      # 200 filter-gen outputs per pixel
WH = 28                    # columns per core (half width)
JP = WH + 4                # padded columns held in SBUF
P_O = 2 * H                # 112 partitions carrying (row, img) pairs
JCH = 7                    # filter-gen j-columns per chunk
N_CORES = 8

F32 = mybir.dt.float32


VERSION = 4

F32R = mybir.dt.float32r
HHEADS = HEADS // 2        # heads per channel-half
NCH = 6                    # PSUM chunks per half
CHF = 96 * WH // NCH       # 448 fp32 per chunk = 16 channels x 28 cols


def build_nc(version=None):
    if version is None:
        version = VERSION
    nc = bacc.Bacc(None)

    xd_in = [
        nc.dram_tensor(f"x_d{k}", [P_O, C, JP], F32, kind="ExternalInput")
        for k in range(K)
    ]
    xg_in = nc.dram_tensor("x_gen", [96, 2, WH, P_O], F32, kind="ExternalInput")
    fw_in = nc.dram_tensor("fw_pk", [96, 2, FCOLS], F32, kind="ExternalInput")
    id_in = nc.dram_tensor("ident", [P_O, P_O], F32R, kind="ExternalInput")
    out_d = nc.dram_tensor("out_sbl", [P_O, C, WH], F32, kind="ExternalOutput")

    with tile.TileContext(nc) as tc:
        with (
            tc.tile_pool(name="big", bufs=1) as big,
            tc.tile_pool(name="sh", bufs=2) as sh,
            tc.tile_pool(name="ps_f", bufs=2, space="PSUM") as ps_f,
            tc.tile_pool(name="ps_a", bufs=NCH, space="PSUM") as ps_a,
        ):
            xd = [
                big.tile([P_O, C, JP], F32, tag=f"xd{k}", name=f"xd{k}")
                for k in range(K)
            ]
            fw_sb = big.tile([96, 2, FCOLS], F32)
            if version >= 4:
                # (kl, h, j) order: conv in1 gets a contiguous innermost dim
                f_sb = big.tile([P_O, K * K, HEADS, WH], F32)
            else:
                f_sb = big.tile([P_O, WH, K * K, HEADS], F32)
            acc = big.tile([P_O, C, WH], F32)
            ident = big.tile([P_O, P_O], F32R)

            nc.sync.dma_start(ident[:], id_in[:])
            for k in range(K):
                nc.sync.dma_start(xd[k][:], xd_in[k][:])
            nc.sync.dma_start(fw_sb[:], fw_in[:])

            # ---- filter generation: f[q, j, kk, h] = sum_c x[c, q, j] * fw[c, kk*8+h]
            for jc in range(WH // JCH):
                xg = sh.tile([96, 2, JCH, P_O], F32, tag="xgprod")
                nc.sync.dma_start(xg[:], xg_in[:, :, jc * JCH : (jc + 1) * JCH, :])
                for jl in range(JCH):
                    j = jc * JCH + jl
                    fps = ps_f.tile([P_O, K * K, HEADS], F32, tag="fps")
                    for ck in range(2):
                        nc.tensor.matmul(
                            fps[:],
                            xg[:, ck, jl, :],      # (96 ch, 112 px) stationary
                            fw_sb[:, ck, :],       # (96 ch, 200)
                            start=(ck == 0),
                            stop=(ck == 1),
                        )
                    if version >= 4:
                        nc.scalar.copy(f_sb[:, :, :, j], fps[:])
                    else:
                        nc.scalar.copy(f_sb[:, j, :, :], fps[:])

            if version == 1:
                # ---- 25 broadcast-mult + accumulate passes, all on DVE
                first = True
                for kh in range(K):
                    for kw in range(K):
                        kl = kh * K + kw
                        xin = xd[kh][:, :, kw : kw + WH]
                        xin4 = xin.rearrange("p (h g) j -> p h g j", h=HEADS)
                        fbc = (
                            f_sb[:, :, kl, :]
                            .transpose([0, 2, 1])
                            .unsqueeze(2)
                            .broadcast_to([P_O, HEADS, CG, WH])
                        )
                        if first:
                            prod = acc
                        else:
                            prod = sh.tile([P_O, C, WH], F32, tag="xgprod")
                        p4 = prod[:].rearrange("p (h g) j -> p h g j", h=HEADS)
                        nc.vector.tensor_mul(p4, xin4, fbc)
                        if not first:
                            nc.vector.tensor_add(acc[:], acc[:], prod[:])
                        first = False
            else:
                # ---- v2/v3: DVE computes the 25 products per channel-half;
                # the PE sums them with float32r identity matmuls
                # accumulating in PSUM (exact fp32), ACT evacuates.
                # v3 additionally gives GPSIMD a column share of the products.
                JS = WH if version != 3 else 18
                for hh in range(2):
                    c0 = hh * 96
                    accps = [
                        ps_a.tile([P_O, CHF], F32, tag="accps", name=f"accps{hh}_{b}")
                        for b in range(NCH)
                    ]
                    for kh in range(K):
                        for kw in range(K):
                            kl = kh * K + kw
                            xin = xd[kh][:, c0 : c0 + 96, kw : kw + WH]
                            xin4 = xin.rearrange("p (h g) j -> p h g j", h=HHEADS)
                            if version >= 4:
                                fbc = (
                                    f_sb[:, kl, hh * HHEADS : (hh + 1) * HHEADS, :]
                                    .unsqueeze(2)
                                    .broadcast_to([P_O, HHEADS, CG, WH])
                                )
                            else:
                                fbc = (
                                    f_sb[:, :, kl, hh * HHEADS : (hh + 1) * HHEADS]
                                    .transpose([0, 2, 1])
                                    .unsqueeze(2)
                                    .broadcast_to([P_O, HHEADS, CG, WH])
                                )
                            prod = sh.tile([P_O, 96, WH], F32R, tag="xgprod")
                            p4 = prod[:].rearrange("p (h g) j -> p h g j", h=HHEADS)
                            if JS >= WH:
                                nc.vector.tensor_mul(p4, xin4, fbc)
                            else:
                                nc.vector.tensor_mul(
                                    p4[:, :, :, :JS],
                                    xin4[:, :, :, :JS],
                                    fbc[:, :, :, :JS],
                                )
                                nc.gpsimd.tensor_mul(
                                    p4[:, :, :, JS:],
                                    xin4[:, :, :, JS:],
                                    fbc[:, :, :, JS:],
                                )
                            pflat = prod[:].rearrange("p c j -> p (c j)")
                            for b in range(NCH):
                                nc.tensor.matmul(
                                    accps[b][:],
                                    ident[:],
                                    pflat[:, b * CHF : (b + 1) * CHF],
                                    start=(kl == 0),
                                    stop=(kl == K * K - 1),
                                )
                    for b in range(NCH):
                        nc.scalar.copy(
                            acc[:, hh * 96 + b * 16 : hh * 96 + (b + 1) * 16, :],
                            accps[b][:].rearrange("p (c j) -> p c j", j=WH),
                        )

            nc.sync.dma_start(out_d[:], acc[:])

    return nc


def shard_inputs(x, filt_w):
    """Split full inputs into the 8 per-core input maps."""
    x = np.ascontiguousarray(np.asarray(x, dtype=np.float32))
    fw = np.ascontiguousarray(np.asarray(filt_w, dtype=np.float32))
    fw_pk = np.ascontiguousarray(fw.reshape(2, 96, FCOLS).transpose(1, 0, 2))

    in_maps = []
    for core in range(N_CORES):
        pair, jh = divmod(core, 2)
        xs = x[2 * pair : 2 * pair + 2]           # (2, C, 56, 56)
        xpad = np.zeros((2, C, H + 4, JP), np.float32)
        lo = jh * WH - 2                           # global col of jp=0
        s_lo, s_hi = max(lo, 0), min(lo + JP, W)
        xpad[:, :, 2 : 2 + H, s_lo - lo : s_lo - lo + (s_hi - s_lo)] = xs[
            :, :, :, s_lo:s_hi
        ]
        m = {"fw_pk": fw_pk, "ident": np.eye(P_O, dtype=np.float32)}
        for k in range(K):
            # x_dk[2*i+img, c, jp] = xpad[img, c, i+k, jp]
            m[f"x_d{k}"] = np.ascontiguousarray(
                xpad[:, :, k : k + H, :].transpose(2, 0, 1, 3).reshape(P_O, C, JP)
            )
        # channel-major copy for filter-gen: x_gen[c96, ck, j, 2*i+img]
        xs_half = xs[:, :, :, jh * WH : (jh + 1) * WH]  # (2, C, 56, 28)
        xg = xs_half.transpose(1, 3, 2, 0).reshape(C, WH, P_O)
        m["x_gen"] = np.ascontiguousarray(
            xg.reshape(2, 96, WH, P_O).transpose(1, 0, 2, 3)
        )
        in_maps.append(m)
    return in_maps


def unshard_output(results):
    """Reassemble the 8 per-core outputs into the full (B, C, H, W) tensor."""
    out = np.empty((B, C, H, W), np.float32)
    for core in range(N_CORES):
        pair, jh = divmod(core, 2)
        arr = np.asarray(results[core]["out_sbl"]).reshape(H, 2, C, WH)
        # arr[i, img, c, j] = out[2*pair+img, c, i, jh*28+j]
        out[2 * pair : 2 * pair + 2, :, :, jh * WH : (jh + 1) * WH] = arr.transpose(
            1, 2, 0, 3
        )
    return out


_NC_CACHE = None


def _get_nc():
    global _NC_CACHE
    if _NC_CACHE is None:
        _NC_CACHE = build_nc()
        if not _NC_CACHE.is_finalized():
            _NC_CACHE.finalize()
    return _NC_CACHE


def run(inputs, trace=False, **kwargs):
    """Run on the 8 NeuronCores; returns BassKernelResults."""
    in_maps = shard_inputs(inputs["x"], inputs["filt_w"])
    nc = _get_nc()
    return run_bass_kernel_spmd(
        nc, in_maps, core_ids=list(range(N_CORES)), trace=trace, **kwargs
    )


def kernel(x, filt_w):
    res = run({"x": x, "filt_w": filt_w})
    return unshard_output(res.results)
